# revision 1
# baseline (speedup 1.0000x reference)
"""Trainium2 Bass kernel for nn_HermesMessageLayer (gnn_message_passing).

Math: out[e,i,n] = sum_{b,f,r,j,m} inp[e,j,m] * precomp[e,f,r]
                                   * kernel[b,f,n,m] * weight[b,r,i,j] + bias[i]

Staging (per core, data-parallel over E across 8 cores):
  KW[(j,m), (f,r,i,n)] = sum_b kernel[b,f,n,m]*weight[b,r,i,j]   (host, tiny)
  t[e, (f,r,i,n)] = inp[e,(j,m)] @ KW                            (PE matmul)
  out[e, (i,n)]   = sum_{fr} precomp[e,fr] * t[e,fr,:] + bias    (DVE/POOL FMAs)

Per 128-edge tile on device:
  - inp rows are transpose-loaded (XBAR DMA, bf16, rows padded 96->128) so the
    contraction dim (j,m) lands on SBUF partitions for the matmul stationary.
  - one PE matmul pair (N=480 x2, two PSUM banks) computes t for 128 edges.
  - ScalarE copies t PSUM->SBUF with bf16 cast (one op, strided over banks).
  - VectorE runs scalar_tensor_tensor FMAs (scalar = per-partition precomp
    column) for fr 0..6 (bias folds into the first); GPSIMD takes fr 7..9 in a
    separate accumulator, merged once per 16-tile group by a batched DVE add.
  - bf16 HWDGE store to a partition-major layout; host un-permutes + upcasts.
"""

import os
import sys

import numpy as np

sys.path.insert(0, "/opt/trn_rl_repo")

import ml_dtypes

import concourse.bass as bass
import concourse.bacc as bacc
import concourse.tile as tile
from concourse import mybir
from concourse.bass_utils import run_bass_kernel_spmd

# Problem dims
E, J, I = 300000, 32, 32
M, N = 3, 3
B, F, R = 6, 5, 2
JM = J * M          # 96
NI = I * N          # 96  (col layout is (i, n): ni = i*3 + n)
FR = F * R          # 10
TCOLS = FR * NI     # 960

NCORES = 8
E_CORE = E // NCORES            # 37500
G = 16                          # tiles per group
TILE_E = 128                    # edges per tile (PSUM partitions)
GROUP_E = G * TILE_E            # 2048
NG = -(-E_CORE // GROUP_E)      # 19 groups
E_PAD = NG * GROUP_E            # 38912

POOL_FRS = 0                    # GPSIMD offload disabled: TensorScalarPtr is not
                                # a legal Pool-engine opcode on TRN2 (walrus
                                # NCC_IXCG966 engine check)

BF16 = mybir.dt.bfloat16
F32 = mybir.dt.float32

_mult = mybir.AluOpType.mult
_add = mybir.AluOpType.add


def build_program(ng: int = NG, pool_frs: int = POOL_FRS):
    """Build the single-core Bass program (same program runs SPMD on all cores)."""
    nc = bacc.Bacc("TRN2", target_bir_lowering=False, debug=False)

    e_pad = ng * GROUP_E
    inp_t = nc.dram_tensor("inp_aug", [e_pad, 128], BF16, kind="ExternalInput").ap()
    pc_t = nc.dram_tensor("pc", [ng, 128, G, FR], F32, kind="ExternalInput").ap()
    kw_t = nc.dram_tensor("kw", [JM, TCOLS], BF16, kind="ExternalInput").ap()
    bias_t = nc.dram_tensor("bias", [128, NI], BF16, kind="ExternalInput").ap()
    out_t = nc.dram_tensor("out", [ng, 128, G, NI], BF16, kind="ExternalOutput").ap()

    n_dve = FR - pool_frs

    with tile.TileContext(nc) as tc:
        with (
            tc.tile_pool(name="const", bufs=1) as const_pool,
            tc.tile_pool(name="inpT", bufs=2) as inpT_pool,
            tc.tile_pool(name="pc", bufs=2) as pc_pool,
            tc.tile_pool(name="tsb", bufs=3) as tsb_pool,
            tc.tile_pool(name="acc", bufs=2) as acc_pool,
            tc.tile_pool(name="upool", bufs=2) as u_pool,
            tc.tile_pool(name="psum", bufs=2, space="PSUM") as psum_pool,
        ):
            kw_sb = const_pool.tile([JM, TCOLS], BF16)
            bias_sb = const_pool.tile([128, NI], BF16)
            nc.sync.dma_start(kw_sb[:], kw_t[:])
            nc.sync.dma_start(bias_sb[:], bias_t[:])

            for g in range(ng):
                inpT = inpT_pool.tile([128, GROUP_E], BF16)
                nc.sync.dma_start(
                    inpT[:],
                    inp_t[g * GROUP_E : (g + 1) * GROUP_E, :],
                    transpose=True,
                )
                pc = pc_pool.tile([128, G, FR], F32)
                nc.sync.dma_start(pc[:], pc_t[g])
                acc = acc_pool.tile([128, G, NI], BF16)
                if pool_frs:
                    u = u_pool.tile([128, G, NI], BF16)

                for gi in range(G):
                    ps = psum_pool.tile([128, 1024], F32)
                    lhsT = inpT[0:JM, gi * TILE_E : (gi + 1) * TILE_E]
                    nc.tensor.matmul(
                        ps[:, 0:480], lhsT, kw_sb[:, 0:480], start=True, stop=True
                    )
                    nc.tensor.matmul(
                        ps[:, 512:992], lhsT, kw_sb[:, 480:960], start=True, stop=True
                    )

                    tsb = tsb_pool.tile([128, TCOLS], BF16)
                    ps_view = ps[:].rearrange("p (b x) -> p b x", b=2)[:, :, 0:480]
                    tsb_view = tsb[:].rearrange("p (b x) -> p b x", b=2)
                    nc.scalar.copy(tsb_view, ps_view)

                    a = acc[:, gi]
                    nc.vector.scalar_tensor_tensor(
                        a,
                        tsb[:, 0:NI],
                        pc[:, gi, 0:1],
                        bias_sb[:],
                        op0=_mult,
                        op1=_add,
                    )
                    for fr in range(1, n_dve):
                        nc.vector.scalar_tensor_tensor(
                            a,
                            tsb[:, fr * NI : (fr + 1) * NI],
                            pc[:, gi, fr : fr + 1],
                            a,
                            op0=_mult,
                            op1=_add,
                        )
                    if pool_frs:
                        ug = u[:, gi]
                        fr0 = n_dve
                        nc.gpsimd.tensor_scalar_mul(
                            ug, tsb[:, fr0 * NI : (fr0 + 1) * NI], pc[:, gi, fr0 : fr0 + 1]
                        )
                        for fr in range(fr0 + 1, FR):
                            nc.gpsimd.scalar_tensor_tensor(
                                ug,
                                tsb[:, fr * NI : (fr + 1) * NI],
                                pc[:, gi, fr : fr + 1],
                                ug,
                                op0=_mult,
                                op1=_add,
                            )

                if pool_frs:
                    nc.vector.tensor_add(acc[:], acc[:], u[:])
                nc.sync.dma_start(out_t[g], acc[:])

    nc.compile()
    return nc


def _pack_core(inp_c, precomp_c, ng: int = NG):
    """Pack one core's slice into the padded/permuted device layouts."""
    e_pad = ng * GROUP_E
    e_c = inp_c.shape[0]
    inp_aug = np.zeros([e_pad, 128], dtype=ml_dtypes.bfloat16)
    inp_aug[:e_c, :JM] = inp_c.reshape(e_c, JM).astype(ml_dtypes.bfloat16)

    pc_pad = np.zeros([e_pad, FR], dtype=np.float32)
    pc_pad[:e_c] = precomp_c.reshape(e_c, FR)
    # tile (g, gi) partition p holds edge g*GROUP_E + gi*TILE_E + p
    pc_perm = np.ascontiguousarray(
        pc_pad.reshape(ng, G, TILE_E, FR).transpose(0, 2, 1, 3)
    )
    return inp_aug, pc_perm


def _pack_shared(kernel, weight, bias):
    # KW[(j,m), (f,r,i,n)] = sum_b kernel[b,f,n,m] * weight[b,r,i,j]
    kw = np.einsum(
        "bfnm,brij->jmfrin",
        kernel.astype(np.float64),
        weight.astype(np.float64),
    ).reshape(JM, TCOLS)
    kw_b = kw.astype(ml_dtypes.bfloat16)
    bias_ni = np.repeat(bias.astype(np.float64), N)  # [NI], ni = i*3+n
    bias_bc = np.tile(bias_ni[None, :], (128, 1)).astype(ml_dtypes.bfloat16)
    return kw_b, bias_bc


_PROGRAM_CACHE = {}


def _get_program(ng: int = NG, pool_frs: int = POOL_FRS):
    key = (ng, pool_frs)
    if key not in _PROGRAM_CACHE:
        _PROGRAM_CACHE[key] = build_program(ng, pool_frs)
    return _PROGRAM_CACHE[key]


def kernel(inp, precomp, kernel, weight, bias):
    inp = np.asarray(inp)
    precomp = np.asarray(precomp)
    kernel_np = np.asarray(kernel)
    weight = np.asarray(weight)
    bias = np.asarray(bias)

    kw_b, bias_bc = _pack_shared(kernel_np, weight, bias)

    in_maps = []
    for c in range(NCORES):
        sl = slice(c * E_CORE, (c + 1) * E_CORE)
        inp_aug, pc_perm = _pack_core(inp[sl], precomp[sl])
        in_maps.append(
            {"inp_aug": inp_aug, "pc": pc_perm, "kw": kw_b, "bias": bias_bc}
        )

    nc = _get_program()
    res = run_bass_kernel_spmd(nc, in_maps, list(range(NCORES)))

    out = np.empty([E, I, N], dtype=np.float32)
    for c in range(NCORES):
        o = np.asarray(res.results[c]["out"]).astype(np.float32)  # [NG,128,G,NI]
        o = o.transpose(0, 2, 1, 3).reshape(NG * GROUP_E, NI)[:E_CORE]
        out[c * E_CORE : (c + 1) * E_CORE] = o.reshape(E_CORE, I, N)
    return out



# revision 3
# speedup vs baseline: 1.6025x; 1.6025x over previous
"""Trainium2 Bass kernel for nn_HermesMessageLayer (gnn_message_passing).

Math: out[e,i,n] = sum_{b,f,r,j,m} inp[e,j,m] * precomp[e,f,r]
                                   * kernel[b,f,n,m] * weight[b,r,i,j] + bias[i]

Staging (per core, data-parallel over E across 8 cores):
  KW[(j,m), (i,n,f,r)] = sum_b kernel[b,f,n,m]*weight[b,r,i,j]   (host, tiny)
  t[e, (ni, fr)] = inp[e,(j,m)] @ KW                             (PE matmul)
  out[e, ni]     = sum_fr pc[e,fr] * t[e, ni, fr]                (DVE, fr innermost)

Key layout choice: fr is the INNERMOST kw column index, so the per-edge
pc contraction vectorizes as ONE DVE tensor_tensor multiply per 16-tile
group (pc broadcast along ni via a stride-0 AP dim; fr stride-1 innermost
keeps the DVE 2x16-bit mode) followed by a short add tree, instead of 10
chained scalar_tensor_tensor ops per tile (no fast mode, serial stalls).

Per 128-edge tile:
  - inp is host-transposed to [jm, E] so group loads are plain contiguous
    DMAs (no XBAR transpose, no 96->128 row pad).
  - one PE matmul pair (480 cols x2, two PSUM banks) computes t'.
  - Scalar engine (3 of 4 tiles) / Pool engine (1 of 4) copies t' PSUM->SBUF
    with fp16 cast.
Per 16-tile group:
  - DVE: prod = t' * pc  (one 2x-mode op), tree adds 5+5 -> 2+2+(1) -> final.
  - Pool: last two (non-2x-eligible) tree levels.
  - fp16 HWDGE store; host un-permutes, upcasts, adds bias.
"""

import sys

import numpy as np

sys.path.insert(0, "/opt/trn_rl_repo")

import concourse.bass as bass
import concourse.bacc as bacc
import concourse.tile as tile
from concourse import mybir
from concourse.bass_utils import run_bass_kernel_spmd

# Problem dims
E, J, I = 300000, 32, 32
M, N = 3, 3
B, F, R = 6, 5, 2
JM = J * M          # 96
NI = I * N          # 96  (ni = i*3 + n)
FR = F * R          # 10  (fr = f*2 + r)
TCOLS = NI * FR     # 960 (col = ni*10 + fr; fr innermost)

NCORES = 8
E_CORE = E // NCORES            # 37500
G = 16                          # tiles per group
TILE_E = 128                    # edges per tile (PSUM partitions)
GROUP_E = G * TILE_E            # 2048
NG = -(-E_CORE // GROUP_E)      # 19 groups
E_PAD = NG * GROUP_E            # 38912

POOL_COPY_EVERY = 0             # GPSIMD cannot access PSUM (BIR verifier) -> ACT does all copies
POOL_TREE = True                # last two tree levels on Pool

F16 = mybir.dt.float16
F32 = mybir.dt.float32

_mult = mybir.AluOpType.mult
_add = mybir.AluOpType.add


def build_program(ng: int = NG):
    """Build the single-core Bass program (same program runs SPMD on all cores)."""
    nc = bacc.Bacc("TRN2", target_bir_lowering=False, debug=False)

    e_pad = ng * GROUP_E
    inpT_t = nc.dram_tensor("inpT", [JM, e_pad], F16, kind="ExternalInput").ap()
    pc_t = nc.dram_tensor("pc", [ng, 128, G, FR], F16, kind="ExternalInput").ap()
    kw_t = nc.dram_tensor("kw", [JM, TCOLS], F16, kind="ExternalInput").ap()
    out_t = nc.dram_tensor("out", [ng, 128, G, NI], F16, kind="ExternalOutput").ap()

    with tile.TileContext(nc) as tc:
        with (
            tc.tile_pool(name="const", bufs=1) as const_pool,
            tc.tile_pool(name="inpT", bufs=2) as inpT_pool,
            tc.tile_pool(name="pc", bufs=2) as pc_pool,
            tc.tile_pool(name="tsb", bufs=2) as tsb_pool,
            tc.tile_pool(name="prod", bufs=2) as prod_pool,
            tc.tile_pool(name="r1", bufs=1) as r1_pool,
            tc.tile_pool(name="r2", bufs=1) as r2_pool,
            tc.tile_pool(name="acc", bufs=2) as acc_pool,
            tc.tile_pool(name="psum", bufs=3, space="PSUM") as psum_pool,
        ):
            kw_sb = const_pool.tile([JM, TCOLS], F16)
            nc.sync.dma_start(kw_sb[:], kw_t[:])

            for g in range(ng):
                inpT = inpT_pool.tile([JM, GROUP_E], F16)
                nc.sync.dma_start(
                    inpT[:], inpT_t[:, g * GROUP_E : (g + 1) * GROUP_E]
                )
                pc = pc_pool.tile([128, G, FR], F16)
                nc.sync.dma_start(pc[:], pc_t[g])

                tsb = tsb_pool.tile([128, G, TCOLS], F16)
                for gi in range(G):
                    ps = psum_pool.tile([128, 1024], F32)
                    lhsT = inpT[:, gi * TILE_E : (gi + 1) * TILE_E]
                    nc.tensor.matmul(
                        ps[:, 0:480], lhsT, kw_sb[:, 0:480], start=True, stop=True
                    )
                    nc.tensor.matmul(
                        ps[:, 512:992], lhsT, kw_sb[:, 480:960], start=True, stop=True
                    )
                    ps_view = ps[:].rearrange("p (b x) -> p b x", b=2)[:, :, 0:480]
                    tsb_view = tsb[:, gi].rearrange("p (b x) -> p b x", b=2)
                    if POOL_COPY_EVERY and gi % POOL_COPY_EVERY == POOL_COPY_EVERY - 1:
                        nc.gpsimd.tensor_copy(out=tsb_view, in_=ps_view)
                    else:
                        nc.scalar.copy(tsb_view, ps_view)

                # prod[p, g, ni, fr] = t'[p, g, ni, fr] * pc[p, g, fr]
                prod = prod_pool.tile([128, G, NI, FR], F16)
                tsb4 = tsb[:].rearrange("p g (ni fr) -> p g ni fr", fr=FR)
                pcb = pc[:, :, None, :].to_broadcast([128, G, NI, FR])
                nc.vector.tensor_tensor(prod[:], tsb4, pcb, _mult)

                # fr-sum tree: 10 -> 5 -> (2,2,1) -> 1
                r1 = r1_pool.tile([128, G, NI, 5], F16)
                nc.vector.tensor_tensor(
                    r1[:], prod[:, :, :, 0:5], prod[:, :, :, 5:10], _add
                )
                r2 = r2_pool.tile([128, G, NI, 2], F16)
                nc.vector.tensor_tensor(
                    r2[:], r1[:, :, :, 0:2], r1[:, :, :, 2:4], _add
                )
                acc = acc_pool.tile([128, G, NI], F16)
                eng = nc.gpsimd if POOL_TREE else nc.vector
                eng.tensor_tensor(acc[:], r2[:, :, :, 0], r2[:, :, :, 1], _add)
                eng.tensor_tensor(acc[:], acc[:], r1[:, :, :, 4], _add)

                nc.sync.dma_start(out_t[g], acc[:])

    nc.compile()
    return nc


def _pack_core(inp_c, precomp_c, ng: int = NG):
    """Pack one core's slice into the padded/permuted device layouts."""
    e_pad = ng * GROUP_E
    e_c = inp_c.shape[0]
    inpT = np.zeros([JM, e_pad], dtype=np.float16)
    inpT[:, :e_c] = inp_c.reshape(e_c, JM).astype(np.float16).T

    pc_pad = np.zeros([e_pad, FR], dtype=np.float16)
    pc_pad[:e_c] = precomp_c.reshape(e_c, FR).astype(np.float16)
    # tile (g, gi) partition p holds edge g*GROUP_E + gi*TILE_E + p
    pc_perm = np.ascontiguousarray(
        pc_pad.reshape(ng, G, TILE_E, FR).transpose(0, 2, 1, 3)
    )
    return inpT, pc_perm


def _pack_shared(kernel, weight, bias=None):
    # KW[(j,m), (i,n,f,r)] = sum_b kernel[b,f,n,m] * weight[b,r,i,j]
    kw = np.einsum(
        "bfnm,brij->jminfr",
        kernel.astype(np.float64),
        weight.astype(np.float64),
    ).reshape(JM, TCOLS)
    return kw.astype(np.float16)


_PROGRAM_CACHE = {}


def _get_program(ng: int = NG):
    key = ng
    if key not in _PROGRAM_CACHE:
        _PROGRAM_CACHE[key] = build_program(ng)
    return _PROGRAM_CACHE[key]


def kernel(inp, precomp, kernel, weight, bias):
    inp = np.asarray(inp)
    precomp = np.asarray(precomp)
    kernel_np = np.asarray(kernel)
    weight = np.asarray(weight)
    bias = np.asarray(bias)

    kw_h = _pack_shared(kernel_np, weight)

    in_maps = []
    for c in range(NCORES):
        sl = slice(c * E_CORE, (c + 1) * E_CORE)
        inpT, pc_perm = _pack_core(inp[sl], precomp[sl])
        in_maps.append({"inpT": inpT, "pc": pc_perm, "kw": kw_h})

    nc = _get_program()
    res = run_bass_kernel_spmd(nc, in_maps, list(range(NCORES)))

    out = np.empty([E, I, N], dtype=np.float32)
    for c in range(NCORES):
        o = np.asarray(res.results[c]["out"]).astype(np.float32)  # [NG,128,G,NI]
        o = o.transpose(0, 2, 1, 3).reshape(NG * GROUP_E, NI)[:E_CORE]
        out[c * E_CORE : (c + 1) * E_CORE] = o.reshape(E_CORE, I, N)
    if bias.any():
        out += bias.astype(np.float32)[None, :, None]
    return out


# revision 5
# speedup vs baseline: 1.6376x; 1.0219x over previous
"""Trainium2 Bass kernel for nn_HermesMessageLayer (gnn_message_passing).

Math: out[e,i,n] = sum_{b,f,r,j,m} inp[e,j,m] * precomp[e,f,r]
                                   * kernel[b,f,n,m] * weight[b,r,i,j] + bias[i]

Staging (per core, data-parallel over E across 8 cores):
  KW[(j,m), (i,n,f,r)] = sum_b kernel[b,f,n,m]*weight[b,r,i,j]   (host, tiny)
  t[e, (ni, fr)] = inp[e,(j,m)] @ KW                             (PE matmul)
  out[e, ni]     = sum_fr pc[e,fr] * t[e, ni, fr]                (DVE, fr innermost)

Key layout choice: fr is the INNERMOST kw column index, so the per-edge
pc contraction vectorizes as ONE DVE tensor_tensor multiply per 16-tile
group (pc broadcast along ni via a stride-0 AP dim; fr stride-1 innermost
keeps the DVE 2x16-bit mode) followed by a short add tree, instead of 10
chained scalar_tensor_tensor ops per tile (no fast mode, serial stalls).

Per 128-edge tile:
  - inp is host-transposed to [jm, E] so group loads are plain contiguous
    DMAs (no XBAR transpose, no 96->128 row pad).
  - one PE matmul pair (480 cols x2, two PSUM banks) computes t'.
  - Scalar engine (3 of 4 tiles) / Pool engine (1 of 4) copies t' PSUM->SBUF
    with fp16 cast.
Per 16-tile group:
  - DVE: prod = t' * pc  (one 2x-mode op), tree adds 5+5 -> 2+2+(1) -> final.
  - Pool: last two (non-2x-eligible) tree levels.
  - fp16 HWDGE store; host un-permutes, upcasts, adds bias.
"""

import sys

import numpy as np

sys.path.insert(0, "/opt/trn_rl_repo")

import concourse.bass as bass
import concourse.bacc as bacc
import concourse.tile as tile
from concourse import mybir
from concourse.bass_utils import run_bass_kernel_spmd

# Problem dims
E, J, I = 300000, 32, 32
M, N = 3, 3
B, F, R = 6, 5, 2
JM = J * M          # 96
NI = I * N          # 96  (ni = i*3 + n)
FR = F * R          # 10  (fr = f*2 + r)
TCOLS = NI * FR     # 960 (col = ni*10 + fr; fr innermost)

NCORES = 8
E_CORE = E // NCORES            # 37500
G = 16                          # tiles per group
TILE_E = 128                    # edges per tile (PSUM partitions)
GROUP_E = G * TILE_E            # 2048
NG = -(-E_CORE // GROUP_E)      # 19 groups
E_PAD = NG * GROUP_E            # 38912

POOL_COPY_EVERY = 0             # GPSIMD cannot access PSUM (BIR verifier) -> ACT does all copies
POOL_TREE = True                # last two tree levels on Pool

F16 = mybir.dt.float16
F32 = mybir.dt.float32

_mult = mybir.AluOpType.mult
_add = mybir.AluOpType.add


def build_program(ng: int = NG):
    """Build the single-core Bass program (same program runs SPMD on all cores)."""
    nc = bacc.Bacc("TRN2", target_bir_lowering=False, debug=False)

    e_pad = ng * GROUP_E
    inpT_t = nc.dram_tensor("inpT", [JM, e_pad], F16, kind="ExternalInput").ap()
    pc_t = nc.dram_tensor("pc", [ng, 128, G, FR], F16, kind="ExternalInput").ap()
    kw_t = nc.dram_tensor("kw", [JM, TCOLS], F16, kind="ExternalInput").ap()
    out_t = nc.dram_tensor("out", [ng, 128, G, NI], F16, kind="ExternalOutput").ap()

    with tile.TileContext(nc) as tc:
        with (
            tc.tile_pool(name="const", bufs=1) as const_pool,
            tc.tile_pool(name="inpT", bufs=2) as inpT_pool,
            tc.tile_pool(name="pc", bufs=2) as pc_pool,
            tc.tile_pool(name="tsb", bufs=2) as tsb_pool,
            tc.tile_pool(name="prod", bufs=2) as prod_pool,
            tc.tile_pool(name="r1", bufs=2) as r1_pool,
            tc.tile_pool(name="r2", bufs=2) as r2_pool,
            tc.tile_pool(name="acc", bufs=2) as acc_pool,
            tc.tile_pool(name="psum", bufs=2, space="PSUM") as psum_pool,
        ):
            kw_sb = const_pool.tile([JM, TCOLS], F16)
            nc.sync.dma_start(kw_sb[:], kw_t[:])

            for g in range(ng):
                inpT = inpT_pool.tile([JM, GROUP_E], F16)
                nc.sync.dma_start(
                    inpT[:], inpT_t[:, g * GROUP_E : (g + 1) * GROUP_E]
                )
                pc = pc_pool.tile([128, G, FR], F16)
                nc.sync.dma_start(pc[:], pc_t[g])

                tsb = tsb_pool.tile([128, G, TCOLS], F16)
                for gp in range(G // 2):
                    # macro PSUM tile: 2 edge-tiles x 2 banks each (8KB = 4 banks)
                    ps = psum_pool.tile([128, 2048], F32)
                    for half in range(2):
                        gi = gp * 2 + half
                        lhsT = inpT[:, gi * TILE_E : (gi + 1) * TILE_E]
                        base = half * 1024
                        nc.tensor.matmul(
                            ps[:, base : base + 480],
                            lhsT,
                            kw_sb[:, 0:480],
                            start=True,
                            stop=True,
                        )
                        nc.tensor.matmul(
                            ps[:, base + 512 : base + 992],
                            lhsT,
                            kw_sb[:, 480:960],
                            start=True,
                            stop=True,
                        )
                    # one ACT copy drains both edge-tiles (4 banks -> 1920 f16)
                    ps_view = ps[:].rearrange("p (b x) -> p b x", b=4)[:, :, 0:480]
                    tsb_view = tsb[:, gp * 2 : gp * 2 + 2].rearrange(
                        "p g (b x) -> p (g b) x", b=2
                    )
                    nc.scalar.copy(tsb_view, ps_view)

                # prod[p, g, ni, fr] = t'[p, g, ni, fr] * pc[p, g, fr]
                prod = prod_pool.tile([128, G, NI, FR], F16)
                tsb4 = tsb[:].rearrange("p g (ni fr) -> p g ni fr", fr=FR)
                pcb = pc[:, :, None, :].to_broadcast([128, G, NI, FR])
                nc.vector.tensor_tensor(prod[:], tsb4, pcb, _mult)

                # fr-sum tree: 10 -> 5 -> (2,2,1) -> 1
                r1 = r1_pool.tile([128, G, NI, 5], F16)
                nc.vector.tensor_tensor(
                    r1[:], prod[:, :, :, 0:5], prod[:, :, :, 5:10], _add
                )
                r2 = r2_pool.tile([128, G, NI, 2], F16)
                nc.vector.tensor_tensor(
                    r2[:], r1[:, :, :, 0:2], r1[:, :, :, 2:4], _add
                )
                acc = acc_pool.tile([128, G, NI], F16)
                eng = nc.gpsimd if POOL_TREE else nc.vector
                eng.tensor_tensor(acc[:], r2[:, :, :, 0], r2[:, :, :, 1], _add)
                eng.tensor_tensor(acc[:], acc[:], r1[:, :, :, 4], _add)

                nc.sync.dma_start(out_t[g], acc[:])

    nc.compile()
    return nc


def _pack_core(inp_c, precomp_c, ng: int = NG):
    """Pack one core's slice into the padded/permuted device layouts."""
    e_pad = ng * GROUP_E
    e_c = inp_c.shape[0]
    inpT = np.zeros([JM, e_pad], dtype=np.float16)
    inpT[:, :e_c] = inp_c.reshape(e_c, JM).astype(np.float16).T

    pc_pad = np.zeros([e_pad, FR], dtype=np.float16)
    pc_pad[:e_c] = precomp_c.reshape(e_c, FR).astype(np.float16)
    # tile (g, gi) partition p holds edge g*GROUP_E + gi*TILE_E + p
    pc_perm = np.ascontiguousarray(
        pc_pad.reshape(ng, G, TILE_E, FR).transpose(0, 2, 1, 3)
    )
    return inpT, pc_perm


def _pack_shared(kernel, weight, bias=None):
    # KW[(j,m), (i,n,f,r)] = sum_b kernel[b,f,n,m] * weight[b,r,i,j]
    kw = np.einsum(
        "bfnm,brij->jminfr",
        kernel.astype(np.float64),
        weight.astype(np.float64),
    ).reshape(JM, TCOLS)
    return kw.astype(np.float16)


_PROGRAM_CACHE = {}


def _get_program(ng: int = NG):
    key = ng
    if key not in _PROGRAM_CACHE:
        _PROGRAM_CACHE[key] = build_program(ng)
    return _PROGRAM_CACHE[key]


def kernel(inp, precomp, kernel, weight, bias):
    inp = np.asarray(inp)
    precomp = np.asarray(precomp)
    kernel_np = np.asarray(kernel)
    weight = np.asarray(weight)
    bias = np.asarray(bias)

    kw_h = _pack_shared(kernel_np, weight)

    in_maps = []
    for c in range(NCORES):
        sl = slice(c * E_CORE, (c + 1) * E_CORE)
        inpT, pc_perm = _pack_core(inp[sl], precomp[sl])
        in_maps.append({"inpT": inpT, "pc": pc_perm, "kw": kw_h})

    nc = _get_program()
    res = run_bass_kernel_spmd(nc, in_maps, list(range(NCORES)))

    out = np.empty([E, I, N], dtype=np.float32)
    for c in range(NCORES):
        o = np.asarray(res.results[c]["out"]).astype(np.float32)  # [NG,128,G,NI]
        o = o.transpose(0, 2, 1, 3).reshape(NG * GROUP_E, NI)[:E_CORE]
        out[c * E_CORE : (c + 1) * E_CORE] = o.reshape(E_CORE, I, N)
    if bias.any():
        out += bias.astype(np.float32)[None, :, None]
    return out


# revision 6
# speedup vs baseline: 1.8330x; 1.1193x over previous
"""Trainium2 Bass kernel for nn_HermesMessageLayer (gnn_message_passing).

Math: out[e,i,n] = sum_{b,f,r,j,m} inp[e,j,m] * precomp[e,f,r]
                                   * kernel[b,f,n,m] * weight[b,r,i,j] + bias[i]

Staging (per core, data-parallel over E across 8 cores):
  KW[(j,m), (i,n,f,r)] = sum_b kernel[b,f,n,m]*weight[b,r,i,j]   (host, tiny)
  t[e, (ni, fr)] = inp[e,(j,m)] @ KW                             (PE matmul)
  out[e, ni]     = sum_fr pc[e,fr] * t[e, ni, fr]                (DVE, fr innermost)

Key layout choice: fr is the INNERMOST kw column index, so the per-edge
pc contraction vectorizes as ONE DVE tensor_tensor multiply per 16-tile
group (pc broadcast along ni via a stride-0 AP dim; fr stride-1 innermost
keeps the DVE 2x16-bit mode) followed by a short add tree, instead of 10
chained scalar_tensor_tensor ops per tile (no fast mode, serial stalls).

Per 128-edge tile:
  - inp is host-transposed to [jm, E] so group loads are plain contiguous
    DMAs (no XBAR transpose, no 96->128 row pad).
  - one PE matmul pair (480 cols x2, two PSUM banks) computes t'.
  - Scalar engine (3 of 4 tiles) / Pool engine (1 of 4) copies t' PSUM->SBUF
    with fp16 cast.
Per 16-tile group:
  - DVE: prod = t' * pc  (one 2x-mode op), tree adds 5+5 -> 2+2+(1) -> final.
  - Pool: last two (non-2x-eligible) tree levels.
  - fp16 HWDGE store; host un-permutes, upcasts, adds bias.
"""

import sys

import numpy as np

sys.path.insert(0, "/opt/trn_rl_repo")

import concourse.bass as bass
import concourse.bacc as bacc
import concourse.tile as tile
from concourse import mybir
from concourse.bass_utils import run_bass_kernel_spmd

# Problem dims
E, J, I = 300000, 32, 32
M, N = 3, 3
B, F, R = 6, 5, 2
JM = J * M          # 96
NI = I * N          # 96  (ni = i*3 + n)
FR = F * R          # 10  (fr = f*2 + r)
TCOLS = NI * FR     # 960 (col = ni*10 + fr; fr innermost)

NCORES = 8
E_CORE = E // NCORES            # 37500
G = 16                          # tiles per group
TILE_E = 128                    # edges per tile (PSUM partitions)
GROUP_E = G * TILE_E            # 2048
NG = -(-E_CORE // GROUP_E)      # 19 groups
E_PAD = NG * GROUP_E            # 38912

POOL_COPY_EVERY = 0             # GPSIMD cannot access PSUM (BIR verifier) -> ACT does all copies
POOL_TREE = False               # Pool TT measured ~9us/op (~4.4ns/elem) and shares
                                # SBUF ports with DVE -> keep the whole tree on DVE

F16 = mybir.dt.float16
F32 = mybir.dt.float32

_mult = mybir.AluOpType.mult
_add = mybir.AluOpType.add


def build_program(ng: int = NG):
    """Build the single-core Bass program (same program runs SPMD on all cores)."""
    nc = bacc.Bacc("TRN2", target_bir_lowering=False, debug=False)

    e_pad = ng * GROUP_E
    inpT_t = nc.dram_tensor("inpT", [JM, e_pad], F16, kind="ExternalInput").ap()
    pc_t = nc.dram_tensor("pc", [ng, 128, G, FR], F16, kind="ExternalInput").ap()
    kw_t = nc.dram_tensor("kw", [JM, TCOLS], F16, kind="ExternalInput").ap()
    out_t = nc.dram_tensor("out", [ng, 128, G, NI], F16, kind="ExternalOutput").ap()

    with tile.TileContext(nc) as tc:
        with (
            tc.tile_pool(name="const", bufs=1) as const_pool,
            tc.tile_pool(name="inpT", bufs=2) as inpT_pool,
            tc.tile_pool(name="pc", bufs=2) as pc_pool,
            tc.tile_pool(name="tsb", bufs=2) as tsb_pool,
            tc.tile_pool(name="prod", bufs=2) as prod_pool,
            tc.tile_pool(name="r1", bufs=2) as r1_pool,
            tc.tile_pool(name="r2", bufs=2) as r2_pool,
            tc.tile_pool(name="acc", bufs=2) as acc_pool,
            tc.tile_pool(name="psum", bufs=2, space="PSUM") as psum_pool,
        ):
            kw_sb = const_pool.tile([JM, TCOLS], F16)
            nc.sync.dma_start(kw_sb[:], kw_t[:])

            for g in range(ng):
                inpT = inpT_pool.tile([JM, GROUP_E], F16)
                nc.sync.dma_start(
                    inpT[:], inpT_t[:, g * GROUP_E : (g + 1) * GROUP_E]
                )
                pc = pc_pool.tile([128, G, FR], F16)
                nc.sync.dma_start(pc[:], pc_t[g])

                tsb = tsb_pool.tile([128, G, TCOLS], F16)
                for gp in range(G // 2):
                    # macro PSUM tile: 2 edge-tiles x 2 banks each (8KB = 4 banks)
                    ps = psum_pool.tile([128, 2048], F32)
                    for half in range(2):
                        gi = gp * 2 + half
                        lhsT = inpT[:, gi * TILE_E : (gi + 1) * TILE_E]
                        base = half * 1024
                        nc.tensor.matmul(
                            ps[:, base : base + 480],
                            lhsT,
                            kw_sb[:, 0:480],
                            start=True,
                            stop=True,
                        )
                        nc.tensor.matmul(
                            ps[:, base + 512 : base + 992],
                            lhsT,
                            kw_sb[:, 480:960],
                            start=True,
                            stop=True,
                        )
                    # one ACT copy drains both edge-tiles (4 banks -> 1920 f16)
                    ps_view = ps[:].rearrange("p (b x) -> p b x", b=4)[:, :, 0:480]
                    tsb_view = tsb[:, gp * 2 : gp * 2 + 2].rearrange(
                        "p g (b x) -> p (g b) x", b=2
                    )
                    nc.scalar.copy(tsb_view, ps_view)

                # prod[p, g, ni, fr] = t'[p, g, ni, fr] * pc[p, g, fr]
                prod = prod_pool.tile([128, G, NI, FR], F16)
                tsb4 = tsb[:].rearrange("p g (ni fr) -> p g ni fr", fr=FR)
                pcb = pc[:, :, None, :].to_broadcast([128, G, NI, FR])
                nc.vector.tensor_tensor(prod[:], tsb4, pcb, _mult)

                # fr-sum tree: 10 -> 5 -> (2,2,1) -> 1
                r1 = r1_pool.tile([128, G, NI, 5], F16)
                nc.vector.tensor_tensor(
                    r1[:], prod[:, :, :, 0:5], prod[:, :, :, 5:10], _add
                )
                r2 = r2_pool.tile([128, G, NI, 2], F16)
                nc.vector.tensor_tensor(
                    r2[:], r1[:, :, :, 0:2], r1[:, :, :, 2:4], _add
                )
                acc = acc_pool.tile([128, G, NI], F16)
                eng = nc.gpsimd if POOL_TREE else nc.vector
                eng.tensor_tensor(acc[:], r2[:, :, :, 0], r2[:, :, :, 1], _add)
                eng.tensor_tensor(acc[:], acc[:], r1[:, :, :, 4], _add)

                nc.sync.dma_start(out_t[g], acc[:])

    nc.compile()
    return nc


def _pack_core(inp_c, precomp_c, ng: int = NG):
    """Pack one core's slice into the padded/permuted device layouts."""
    e_pad = ng * GROUP_E
    e_c = inp_c.shape[0]
    inpT = np.zeros([JM, e_pad], dtype=np.float16)
    inpT[:, :e_c] = inp_c.reshape(e_c, JM).astype(np.float16).T

    pc_pad = np.zeros([e_pad, FR], dtype=np.float16)
    pc_pad[:e_c] = precomp_c.reshape(e_c, FR).astype(np.float16)
    # tile (g, gi) partition p holds edge g*GROUP_E + gi*TILE_E + p
    pc_perm = np.ascontiguousarray(
        pc_pad.reshape(ng, G, TILE_E, FR).transpose(0, 2, 1, 3)
    )
    return inpT, pc_perm


def _pack_shared(kernel, weight, bias=None):
    # KW[(j,m), (i,n,f,r)] = sum_b kernel[b,f,n,m] * weight[b,r,i,j]
    kw = np.einsum(
        "bfnm,brij->jminfr",
        kernel.astype(np.float64),
        weight.astype(np.float64),
    ).reshape(JM, TCOLS)
    return kw.astype(np.float16)


_PROGRAM_CACHE = {}


def _get_program(ng: int = NG):
    key = ng
    if key not in _PROGRAM_CACHE:
        _PROGRAM_CACHE[key] = build_program(ng)
    return _PROGRAM_CACHE[key]


def kernel(inp, precomp, kernel, weight, bias):
    inp = np.asarray(inp)
    precomp = np.asarray(precomp)
    kernel_np = np.asarray(kernel)
    weight = np.asarray(weight)
    bias = np.asarray(bias)

    kw_h = _pack_shared(kernel_np, weight)

    in_maps = []
    for c in range(NCORES):
        sl = slice(c * E_CORE, (c + 1) * E_CORE)
        inpT, pc_perm = _pack_core(inp[sl], precomp[sl])
        in_maps.append({"inpT": inpT, "pc": pc_perm, "kw": kw_h})

    nc = _get_program()
    res = run_bass_kernel_spmd(nc, in_maps, list(range(NCORES)))

    out = np.empty([E, I, N], dtype=np.float32)
    for c in range(NCORES):
        o = np.asarray(res.results[c]["out"]).astype(np.float32)  # [NG,128,G,NI]
        o = o.transpose(0, 2, 1, 3).reshape(NG * GROUP_E, NI)[:E_CORE]
        out[c * E_CORE : (c + 1) * E_CORE] = o.reshape(E_CORE, I, N)
    if bias.any():
        out += bias.astype(np.float32)[None, :, None]
    return out
